# revision 24
# baseline (speedup 1.0000x reference)
"""Causal self-attention (QKV GEMM + RoPE + causal softmax attention + output
projection) for Trainium2, sharded over 8 NeuronCores.

Sharding: tensor-parallel over heads (2 heads/core). Each core computes the
QKV projections for its heads (full token range), RoPE, causal attention, and
a partial output projection over its heads' channels; the host sums the 8
partial projections (the only cross-core reduction) and reshapes.

Matmul operands are fp16 (full-rate PE with hidden weight loads); all
accumulation is fp32 in PSUM, softmax statistics are fp32.

This version fuses the phases into one software-pipelined stream so the
PE never drains between them:
- phase A is emitted as 32 single-m-tile windows (16 QKV k-tiles each);
  RoPE / V-eviction / q,k-transposes of window m are deferred into window
  m+1 so they never stall the PE.
- attention is emitted as head-interleaved query-chunk pairs spliced
  between A windows as soon as their qkT tiles exist; the Scalar-bound
  exp work overlaps the PE-bound GEMM windows.
- softmax denominators: fp16 at-tile adds on DVE, one gpsimd
  partition_all_reduce per chunk (no PE, no extra PSUM), reciprocal and
  scale on DVE, pipelined one A-window behind.
- the output projection is spliced into the attention tail and reuses
  the A/B PSUM pools (everything fits the 8 banks).
- causal narrowing: on diagonal key tiles all ops run only on the valid
  [qlo:] columns, with one shared [128,128] triangle mask.
- all matmul operands are converted to fp16 on the host; x is laid out
  in DMA-issue order (contiguous per partition per window); output
  partials are fp16 and the host accumulates in fp32.
"""

import os
import sys

import numpy as np


def _ensure_concourse():
    try:
        import concourse.bass  # noqa: F401
        return
    except ImportError:
        pass
    for p in (
        "/opt/trn_rl_repo",
        os.path.expanduser("~/.axon_site/_ro/trn_rl_repo"),
        "/root/.axon_site/_ro/trn_rl_repo",
    ):
        if os.path.isdir(p) and p not in sys.path:
            sys.path.insert(0, p)
    import concourse.bass  # noqa: F401


# Problem shape (hardcoded per contract)
B, T, C, H = 2, 2048, 2048, 16
D, RD = 128, 64
NCORES = 8
HPC = H // NCORES          # heads per core = 2
BT = B * T                 # 4096
P = 128
MT = T // P                # 16 token tiles per batch
KTC = C // P               # 16 contraction tiles over C
FPC = 3 * HPC * D          # 768 qkv features per core
NQ = 512                   # query chunk
NJ = T // NQ               # 4 query chunks per instance
SCALE = 1.0 / float(np.sqrt(D))

_PROGRAM = None


def _build_program():
    _ensure_concourse()
    from contextlib import ExitStack

    import concourse.bacc as bacc
    import concourse.mybir as mybir
    import concourse.tile as tile
    from concourse import bass_isa
    from concourse.alu_op_type import AluOpType
    from concourse.masks import make_identity

    F32 = mybir.dt.float32
    MMDT = mybir.dt.float16
    EXP = mybir.ActivationFunctionType.Exp
    MUL = AluOpType.mult
    SUB = AluOpType.subtract
    ADD = AluOpType.add
    PSUM = "PSUM"

    nc = bacc.Bacc("TRN2", target_bir_lowering=False, debug=False,
                   num_devices=NCORES)

    xt_d = nc.dram_tensor("xt", [P, BT * KTC], MMDT, kind="ExternalInput").ap()
    w_d = nc.dram_tensor("wqkv", [P, KTC * FPC], MMDT, kind="ExternalInput").ap()
    cos_d = nc.dram_tensor("cosw", [P, (BT // P) * RD], F32, kind="ExternalInput").ap()
    sin_d = nc.dram_tensor("sinw", [P, (BT // P) * RD], F32, kind="ExternalInput").ap()
    msk_d = nc.dram_tensor("maskd", [P, P], MMDT, kind="ExternalInput").ap()
    wp_d = nc.dram_tensor("wproj", [P, HPC * C], MMDT, kind="ExternalInput").ap()
    out_d = nc.dram_tensor("outp", [BT, C], MMDT, kind="ExternalOutput").ap()

    WQ = KTC * FPC // 4        # qkv weight quarter, 4 k-tiles each

    with tile.TileContext(nc) as tc, ExitStack() as gctx:
        ep = gctx.enter_context

        const = ep(tc.tile_pool(name="const", bufs=1))
        msk_sb = const.tile([P, P], MMDT, tag="msk")
        cos_sb = const.tile([P, (BT // P) * RD], F32, tag="cos")
        sin_sb = const.tile([P, (BT // P) * RD], F32, tag="sin")
        ident = const.tile([P, P], MMDT, tag="ident")
        wp_sb = const.tile([P, HPC * C], MMDT, tag="wp")

        # consts on the idle SP queue, off the critical path
        nc.sync.dma_start(out=msk_sb[:], in_=msk_d)
        nc.sync.dma_start(out=cos_sb[:], in_=cos_d)
        nc.sync.dma_start(out=sin_sb[:], in_=sin_d)
        nc.sync.dma_start(out=wp_sb[:], in_=wp_d)
        make_identity(nc, ident[:])

        qkt_pool = ep(tc.tile_pool(name="qkt", bufs=2))
        v_pool = ep(tc.tile_pool(name="v", bufs=2))
        yt_pool = ep(tc.tile_pool(name="yt", bufs=1))
        yt_all = yt_pool.tile([P, B * HPC * T], MMDT, tag="yt")
        xcol = ep(tc.tile_pool(name="xcol", bufs=3))
        rotp = ep(tc.tile_pool(name="rot", bufs=3))
        tmpp = ep(tc.tile_pool(name="tmp", bufs=2))
        attnp = ep(tc.tile_pool(name="attn", bufs=4))
        saccp = ep(tc.tile_pool(name="sacc", bufs=4))
        srepp = ep(tc.tile_pool(name="srep", bufs=2))
        rrepp = ep(tc.tile_pool(name="rrep", bufs=2))
        outrow = ep(tc.tile_pool(name="orow", bufs=3))

        # PSUM: exactly 8 banks
        ps5 = ep(tc.tile_pool(name="ps5", bufs=2, space=PSUM))   # qk gemm
        ps2 = ep(tc.tile_pool(name="ps2", bufs=1, space=PSUM))   # v gemm
        pst = ep(tc.tile_pool(name="pst", bufs=1, space=PSUM))   # transposes
        pss = ep(tc.tile_pool(name="pss", bufs=2, space=PSUM))   # scores
        psy = ep(tc.tile_pool(name="psy", bufs=2, space=PSUM))   # attn out

        # x chunk prefetching, one [P, KTC, P] chunk per A window
        prefetched = {}

        def fetch_x(b, m):
            key = (b, m)
            if key in prefetched:
                return prefetched.pop(key)
            xo = (b * MT + m) * KTC * P
            xc = xcol.tile([P, KTC, P], MMDT, tag="xc")
            # issue on the SP queue: the gpsimd queue carries the long
            # partition_all_reduce calls and must not delay x loads
            nc.sync.dma_start(
                out=xc[:],
                in_=xt_d[:, xo:xo + KTC * P].rearrange(
                    "p (k t) -> p k t", k=KTC))
            return xc

        def prefetch_x(b, m):
            prefetched[(b, m)] = fetch_x(b, m)

        prefetch_x(0, 0)

        wstack = ExitStack()
        wpool = wstack.enter_context(tc.tile_pool(name="wqkv", bufs=1))
        w_sbs = [wpool.tile([P, WQ], MMDT, tag=f"w{q}", name=f"w{q}")
                 for q in range(4)]
        for q in range(4):
            nc.gpsimd.dma_start(out=w_sbs[q][:], in_=w_d[:, q * WQ:(q + 1) * WQ])

        def wslice(kt, lo, hi):
            return w_sbs[kt // 4][:, (kt % 4) * FPC + lo:(kt % 4) * FPC + hi]

        qkts = {}
        v_sbs = {}
        # deferred per-window epilogue: (b, m, p2, rot)
        pending_a = [None]

        def flush_a():
            if pending_a[0] is None:
                return
            b, m, p2, rot = pending_a[0]
            pending_a[0] = None
            # V eviction (frees ps2 for this window's mm256 sweep)
            nc.scalar.copy(v_sbs[b][:, m * HPC * D:(m + 1) * HPC * D], p2[:])
            # transpose rotated q,k into [d, t]; 4 tiles share one bank
            tp4 = pst.tile([P, 4, P], MMDT, tag="tp", name=f"tp_{b}_{m}")
            for hb in range(4):
                nc.tensor.transpose(tp4[:, hb, :], rot[:, hb * P:(hb + 1) * P],
                                    ident[:])
            qv = qkts[b][:].rearrange("p (hb t) -> p hb t", hb=4)
            nc.scalar.copy(qv[:, :, m * P:(m + 1) * P], tp4[:])

        def emit_a(b, m):
            if m == 0:
                qkts[b] = qkt_pool.tile([P, 4 * T], MMDT, tag="qkT",
                                        name=f"qkT_{b}")
                v_sbs[b] = v_pool.tile([P, MT * HPC * D], MMDT, tag="v",
                                       name=f"v_{b}")
            nm = (b, m + 1) if m + 1 < MT else (b + 1, 0)
            if nm[0] < B:
                prefetch_x(*nm)
            xc = fetch_x(b, m)
            p5 = ps5.tile([P, 512], F32, tag="p5", name=f"p5_{b}_{m}")
            p2 = ps2.tile([P, 256], F32, tag="p2", name=f"p2_{b}_{m}")
            for kt in range(KTC):
                nc.tensor.matmul(p5[:], xc[:, kt, :], wslice(kt, 0, 512),
                                 start=(kt == 0), stop=(kt == KTC - 1))
            # previous window's epilogue lands here: its RoPE is done, so
            # the PE transposes never wait, and ps2/pst free up in time
            flush_a()
            # RoPE on the q|k half, writes rot
            gm = b * MT + m
            rot = rotp.tile([P, 512], MMDT, tag="rot", name=f"rot_{b}_{m}")
            p3 = p5[:].rearrange("p (blk two d) -> p blk two d", two=2, d=RD)
            re_, im_ = p3[:, :, 0, :], p3[:, :, 1, :]
            r3 = rot[:].rearrange("p (blk two d) -> p blk two d", two=2, d=RD)
            cosb = (cos_sb[:, gm * RD:(gm + 1) * RD]
                    .unsqueeze(1).broadcast_to([P, 4, RD]))
            sinb = (sin_sb[:, gm * RD:(gm + 1) * RD]
                    .unsqueeze(1).broadcast_to([P, 4, RD]))
            t1 = tmpp.tile([P, 256], F32, tag="t1")
            t2 = tmpp.tile([P, 256], F32, tag="t2")
            t1v = t1[:].rearrange("p (blk d) -> p blk d", d=RD)
            t2v = t2[:].rearrange("p (blk d) -> p blk d", d=RD)
            nc.vector.tensor_tensor(t1v, re_, cosb, MUL)
            nc.vector.tensor_tensor(t2v, im_, sinb, MUL)
            nc.vector.tensor_tensor(r3[:, :, 0, :], t1v, t2v, SUB)
            t3 = tmpp.tile([P, 256], F32, tag="t3")
            t4 = tmpp.tile([P, 256], F32, tag="t4")
            t3v = t3[:].rearrange("p (blk d) -> p blk d", d=RD)
            t4v = t4[:].rearrange("p (blk d) -> p blk d", d=RD)
            nc.vector.tensor_tensor(t3v, re_, sinb, MUL)
            nc.vector.tensor_tensor(t4v, im_, cosb, MUL)
            nc.vector.tensor_tensor(r3[:, :, 1, :], t3v, t4v, ADD)
            # V projection sweep after the epilogue so ps2 (bufs=1) is free
            for kt in range(KTC):
                nc.tensor.matmul(p2[:], xc[:, kt, :], wslice(kt, 512, FPC),
                                 start=(kt == 0), stop=(kt == KTC - 1))
            pending_a[0] = (b, m, p2, rot)

        # ---- attention chunk pairs (both heads, interleaved) ----
        chunk_st = {}

        def emit_p(b, j):
            flush_a()
            qkT = qkts[b]
            v_sb = v_sbs[b]
            nkt = 4 * (j + 1)
            y_ps = {}
            sacc = {}
            for h in range(HPC):
                y_ps[h] = psy.tile([P, NQ], F32, tag="y", name=f"y_{b}_{j}_{h}")
                sacc[h] = saccp.tile([P, NQ], MMDT, tag="sa",
                                     name=f"sa_{b}_{j}_{h}")
            for kt in range(nkt):
                ktl = kt - (nkt - 4)
                qlo = max(ktl, 0) * P
                for h in range(HPC):
                    sc = pss.tile([P, NQ], F32, tag="sc",
                                  name=f"sc_{b}_{j}_{kt}_{h}")
                    nc.tensor.matmul(
                        sc[:, qlo:],
                        qkT[:, (2 + h) * T + kt * P:(2 + h) * T + (kt + 1) * P],
                        qkT[:, h * T + j * NQ + qlo: h * T + (j + 1) * NQ],
                        start=True, stop=True)
                    at = attnp.tile([P, NQ], MMDT, tag="at",
                                    name=f"at_{b}_{j}_{kt}_{h}")
                    nc.scalar.activation(at[:, qlo:], sc[:, qlo:], EXP,
                                         scale=SCALE)
                    if ktl >= 0:
                        nc.vector.tensor_tensor(
                            at[:, qlo:qlo + P], at[:, qlo:qlo + P],
                            msk_sb[:], MUL)
                    if kt == 0:
                        nc.vector.tensor_copy(sacc[h][:], at[:])
                    else:
                        nc.vector.tensor_tensor(sacc[h][:, qlo:],
                                                sacc[h][:, qlo:],
                                                at[:, qlo:], ADD)
                    nc.tensor.matmul(
                        y_ps[h][:, qlo:],
                        v_sb[:, kt * HPC * D + h * D:kt * HPC * D + (h + 1) * D],
                        at[:, qlo:], start=(kt == 0), stop=(kt == nkt - 1),
                        skip_group_check=True)
            chunk_st[(b, j)] = (y_ps, sacc)

        def emit_f1(b, j):
            # denominator partition reduce, issued right after the pair so
            # the 3.5us gpsimd calls drain while the next A windows run
            y_ps, sacc = chunk_st.pop((b, j))
            sreps = {}
            for h in range(HPC):
                srep = srepp.tile([P, NQ], F32, tag="sr",
                                  name=f"sr_{b}_{j}_{h}")
                nc.gpsimd.partition_all_reduce(srep[:], sacc[h][:], P,
                                               bass_isa.ReduceOp.add)
                sreps[h] = srep
            chunk_st[(b, j)] = (y_ps, sreps)

        def emit_f2(b, j):
            # normalization, emitted two windows later: the reduces are
            # done by now so the DVE queue never blocks on gpsimd
            y_ps, sreps = chunk_st.pop((b, j))
            inst0 = b * HPC
            for h in range(HPC):
                rrep = rrepp.tile([P, NQ], F32, tag="rr",
                                  name=f"rr_{b}_{j}_{h}")
                with nc.allow_low_precision(reason="softmax recip"):
                    nc.vector.reciprocal_approx_fast(out=rrep[:],
                                                     in_=sreps[h][:])
                nc.vector.tensor_tensor(
                    yt_all[:, (inst0 + h) * T + j * NQ:
                           (inst0 + h) * T + (j + 1) * NQ],
                    y_ps[h][:], rrep[:], MUL)

        # ---- output projection m-tile (reuses the A/B psum banks) ----
        c_cnt = [0]
        C_POOLS = [(None, "p5"), (None, "sc"), (None, "y")]

        def emit_c(b, m):
            flush_a()
            cpools = [(ps5, "p5"), (pss, "sc"), (psy, "y")]
            orow = outrow.tile([P, C], MMDT, tag="orow")
            for oc in range(4):
                if c_cnt[0] < 8:
                    # early tiles overlap the draining final attention
                    # pair; only ps5 is reliably free then
                    pool, tg = ps5, "p5"
                else:
                    pool, tg = cpools[c_cnt[0] % 3]
                c_cnt[0] += 1
                op = pool.tile([P, 512], F32, tag=tg, name=f"op_{b}_{m}_{oc}")
                for h in range(HPC):
                    nc.tensor.matmul(
                        op[:],
                        yt_all[:, (b * HPC + h) * T + m * P:
                               (b * HPC + h) * T + (m + 1) * P],
                        wp_sb[:, h * C + oc * 512: h * C + (oc + 1) * 512],
                        start=(h == 0), stop=(h == HPC - 1))
                # split each eviction across both engines so the depth-2
                # ps5 rotation never gates the PE
                nc.scalar.copy(orow[:, oc * 512:oc * 512 + 256],
                               op[:, 0:256])
                nc.vector.tensor_copy(orow[:, oc * 512 + 256:(oc + 1) * 512],
                                      op[:, 256:512])
            nc.sync.dma_start(
                out=out_d[(b * MT + m) * P:(b * MT + m + 1) * P, :],
                in_=orow[:])

        # ---- the fused schedule ----
        sched = []
        for b in range(B):
            for m in range(MT):
                sched.append(("A", b, m))
                if b > 0 and m == 0:
                    sched += [("P", b - 1, 3), ("F1", b - 1, 3)]
                if b > 0 and m == 2:
                    sched.append(("F2", b - 1, 3))
                if m in (4, 8, 12):
                    sched += [("P", b, m // 4 - 1), ("F1", b, m // 4 - 1)]
                if m in (6, 10, 14):
                    sched.append(("F2", b, m // 4 - 1))
        bl = B - 1
        sched += [("P", bl, 3), ("F1", bl, 3), ("C", 0, 0), ("F2", bl, 3)]
        for m in range(1, MT):
            sched.append(("C", 0, m))
        for m in range(MT):
            sched.append(("C", 1, m))

        emitters = {"A": emit_a, "P": emit_p, "F1": emit_f1, "F2": emit_f2,
                    "C": emit_c}
        for kind, b, i in sched:
            emitters[kind](b, i)

        wstack.close()

    nc.compile()
    return nc


def _perm(rows):
    return np.concatenate([rows[0::2], rows[1::2]], axis=0)


def _host_inputs(x, mask, freqs_cos, freqs_sin, w_attn, w_proj):
    f32 = np.float32
    f16 = np.float16
    x = np.asarray(x, f32)
    fc = np.asarray(freqs_cos, f32)
    fs = np.asarray(freqs_sin, f32)
    w_attn = np.asarray(w_attn, f32)
    w_proj = np.asarray(w_proj, f32)

    # x in DMA-issue order: per partition, contiguous [b][m][kt][tok]
    Xv = x.reshape(B, MT, P, KTC, P).transpose(4, 0, 1, 3, 2)
    # [p, b, m, kt, tok]
    xt_host = np.ascontiguousarray(Xv.reshape(P, -1)).astype(f16)

    def rows_arrange(a):  # [BT, RD] -> [P, (BT//P)*RD]
        return np.ascontiguousarray(
            a.reshape(BT // P, P, RD).transpose(1, 0, 2).reshape(P, -1))

    cosw = rows_arrange(np.concatenate([fc] * B, axis=0))
    sinw = rows_arrange(np.concatenate([fs] * B, axis=0))

    # one [k, q] triangle (attend iff k <= q) covers every diagonal subtile
    maskd = np.ascontiguousarray(np.triu(np.ones((P, P), dtype=f16)))

    wq, wk, wv = w_attn[0:C], w_attn[C:2 * C], w_attn[2 * C:3 * C]
    in_maps = []
    for c in range(NCORES):
        h0, h1 = HPC * c, HPC * c + 1
        Wc = np.concatenate([
            _perm(wq[h0 * D:(h0 + 1) * D]), _perm(wq[h1 * D:(h1 + 1) * D]),
            _perm(wk[h0 * D:(h0 + 1) * D]), _perm(wk[h1 * D:(h1 + 1) * D]),
            wv[h0 * D:(h0 + 1) * D], wv[h1 * D:(h1 + 1) * D]], axis=0)
        wqkv_c = np.ascontiguousarray(
            Wc.T.reshape(KTC, P, FPC).transpose(1, 0, 2).reshape(P, KTC * FPC)
        ).astype(f16)
        wp_c = w_proj[:, c * HPC * D:(c + 1) * HPC * D].T  # [256, C]
        wp_c = np.ascontiguousarray(
            wp_c.reshape(HPC, P, C).transpose(1, 0, 2).reshape(P, HPC * C)
        ).astype(f16)
        in_maps.append({
            "xt": xt_host, "wqkv": wqkv_c, "cosw": cosw, "sinw": sinw,
            "maskd": maskd, "wproj": wp_c,
        })
    return in_maps


def kernel(x, mask, freqs_cos, freqs_sin, w_attn, w_proj):
    global _PROGRAM
    _ensure_concourse()
    from concourse.bass_utils import run_bass_kernel_spmd

    if _PROGRAM is None:
        _PROGRAM = _build_program()
    nc = _PROGRAM

    in_maps = _host_inputs(x, mask, freqs_cos, freqs_sin, w_attn, w_proj)
    res = run_bass_kernel_spmd(nc, in_maps, list(range(NCORES)))
    out = res.results[0]["outp"].astype(np.float32)
    for i in range(1, NCORES):
        out = out + res.results[i]["outp"].astype(np.float32)
    return np.ascontiguousarray(out.reshape(B, T, C))


# revision 28
# speedup vs baseline: 1.2218x; 1.2218x over previous
"""Causal self-attention (QKV GEMM + RoPE + causal softmax attention + output
projection) for Trainium2, sharded over 8 NeuronCores.

Sharding: tensor-parallel over heads (2 heads/core). Each core computes the
QKV projections for its heads (full token range), RoPE, causal attention, and
a partial output projection over its heads' channels; the host sums the 8
partial projections (the only cross-core reduction) and reshapes.

Matmul operands are fp16 (full-rate PE with hidden weight loads); all
accumulation is fp32 in PSUM, softmax statistics are fp32.

This version fuses the phases into one software-pipelined stream so the
PE never drains between them:
- phase A is emitted as 32 single-m-tile windows (16 QKV k-tiles each);
  RoPE / V-eviction / q,k-transposes of window m are deferred into window
  m+1 so they never stall the PE.
- attention is emitted as head-interleaved query-chunk pairs spliced
  between A windows as soon as their qkT tiles exist; the Scalar-bound
  exp work overlaps the PE-bound GEMM windows.
- softmax denominators: fp16 at-tile adds on DVE, one gpsimd
  partition_all_reduce per chunk (no PE, no extra PSUM), reciprocal and
  scale on DVE, pipelined one A-window behind.
- the output projection is spliced into the attention tail and reuses
  the A/B PSUM pools (everything fits the 8 banks).
- causal narrowing: on diagonal key tiles all ops run only on the valid
  [qlo:] columns, with one shared [128,128] triangle mask.
- all matmul operands are converted to fp16 on the host; x is laid out
  in DMA-issue order (contiguous per partition per window); output
  partials are fp16 and the host accumulates in fp32.
"""

import os
import sys

import numpy as np


def _ensure_concourse():
    try:
        import concourse.bass  # noqa: F401
        return
    except ImportError:
        pass
    for p in (
        "/opt/trn_rl_repo",
        os.path.expanduser("~/.axon_site/_ro/trn_rl_repo"),
        "/root/.axon_site/_ro/trn_rl_repo",
    ):
        if os.path.isdir(p) and p not in sys.path:
            sys.path.insert(0, p)
    import concourse.bass  # noqa: F401


# Problem shape (hardcoded per contract)
B, T, C, H = 2, 2048, 2048, 16
D, RD = 128, 64
NCORES = 8
HPC = H // NCORES          # heads per core = 2
BT = B * T                 # 4096
P = 128
MT = T // P                # 16 token tiles per batch
KTC = C // P               # 16 contraction tiles over C
FPC = 3 * HPC * D          # 768 qkv features per core
NQ = 512                   # query chunk
NJ = T // NQ               # 4 query chunks per instance
SCALE = 1.0 / float(np.sqrt(D))

_PROGRAM = None


def _build_program():
    _ensure_concourse()
    from contextlib import ExitStack

    import concourse.bacc as bacc
    import concourse.mybir as mybir
    import concourse.tile as tile
    from concourse import bass_isa
    from concourse.alu_op_type import AluOpType
    from concourse.masks import make_identity

    F32 = mybir.dt.float32
    MMDT = mybir.dt.float16
    EXP = mybir.ActivationFunctionType.Exp
    MUL = AluOpType.mult
    SUB = AluOpType.subtract
    ADD = AluOpType.add
    PSUM = "PSUM"

    nc = bacc.Bacc("TRN2", target_bir_lowering=False, debug=False,
                   num_devices=NCORES)

    xt_d = nc.dram_tensor("xt", [P, BT * KTC], MMDT, kind="ExternalInput").ap()
    w_d = nc.dram_tensor("wqkv", [P, KTC * FPC], MMDT, kind="ExternalInput").ap()
    cos_d = nc.dram_tensor("cosw", [P, (BT // P) * RD], F32, kind="ExternalInput").ap()
    sin_d = nc.dram_tensor("sinw", [P, (BT // P) * RD], F32, kind="ExternalInput").ap()
    msk_d = nc.dram_tensor("maskd", [P, P], MMDT, kind="ExternalInput").ap()
    wp_d = nc.dram_tensor("wproj", [P, HPC * C], MMDT, kind="ExternalInput").ap()
    out_d = nc.dram_tensor("outp", [BT, C], MMDT, kind="ExternalOutput").ap()

    WQ = KTC * FPC // 4        # qkv weight quarter, 4 k-tiles each

    with tile.TileContext(nc) as tc, ExitStack() as gctx:
        ep = gctx.enter_context

        const = ep(tc.tile_pool(name="const", bufs=1))
        msk_sb = const.tile([P, P], MMDT, tag="msk")
        cos_sb = const.tile([P, (BT // P) * RD], F32, tag="cos")
        sin_sb = const.tile([P, (BT // P) * RD], F32, tag="sin")
        ident = const.tile([P, P], MMDT, tag="ident")
        wp_sb = const.tile([P, HPC * C], MMDT, tag="wp")

        # consts on the idle SP queue, off the critical path
        nc.sync.dma_start(out=msk_sb[:], in_=msk_d)
        nc.sync.dma_start(out=cos_sb[:], in_=cos_d)
        nc.sync.dma_start(out=sin_sb[:], in_=sin_d)
        nc.sync.dma_start(out=wp_sb[:], in_=wp_d)
        make_identity(nc, ident[:])

        qkt_pool = ep(tc.tile_pool(name="qkt", bufs=2))
        v_pool = ep(tc.tile_pool(name="v", bufs=2))
        yt_pool = ep(tc.tile_pool(name="yt", bufs=1))
        yt_all = yt_pool.tile([P, B * HPC * T], MMDT, tag="yt")
        xcol = ep(tc.tile_pool(name="xcol", bufs=4))
        rotp = ep(tc.tile_pool(name="rot", bufs=3))
        tmpp = ep(tc.tile_pool(name="tmp", bufs=2))
        attnp = ep(tc.tile_pool(name="attn", bufs=4))
        saccp = ep(tc.tile_pool(name="sacc", bufs=4))
        srepp = ep(tc.tile_pool(name="srep", bufs=2))
        rrepp = ep(tc.tile_pool(name="rrep", bufs=2))
        outrow = ep(tc.tile_pool(name="orow", bufs=3))

        # PSUM: exactly 8 banks
        ps5 = ep(tc.tile_pool(name="ps5", bufs=2, space=PSUM))   # qk gemm
        ps2 = ep(tc.tile_pool(name="ps2", bufs=1, space=PSUM))   # v gemm
        pst = ep(tc.tile_pool(name="pst", bufs=1, space=PSUM))   # transposes
        pss = ep(tc.tile_pool(name="pss", bufs=2, space=PSUM))   # scores
        psy = ep(tc.tile_pool(name="psy", bufs=2, space=PSUM))   # attn out

        # x chunk prefetching, one [P, KTC, P] chunk per A window
        prefetched = {}

        def fetch_x(b, m):
            key = (b, m)
            if key in prefetched:
                return prefetched.pop(key)
            xo = (b * MT + m) * KTC * P
            xc = xcol.tile([P, KTC, P], MMDT, tag="xc")
            # issue on the Act queue: the gpsimd queue carries the long
            # partition_all_reduce calls and must not delay x loads
            nc.scalar.dma_start(
                out=xc[:],
                in_=xt_d[:, xo:xo + KTC * P].rearrange(
                    "p (k t) -> p k t", k=KTC))
            return xc

        def prefetch_x(b, m):
            prefetched[(b, m)] = fetch_x(b, m)

        prefetch_x(0, 0)

        wstack = ExitStack()
        wpool = wstack.enter_context(tc.tile_pool(name="wqkv", bufs=1))
        w_sbs = [wpool.tile([P, WQ], MMDT, tag=f"w{q}", name=f"w{q}")
                 for q in range(4)]
        for q in range(4):
            nc.sync.dma_start(out=w_sbs[q][:], in_=w_d[:, q * WQ:(q + 1) * WQ])

        def wslice(kt, lo, hi):
            return w_sbs[kt // 4][:, (kt % 4) * FPC + lo:(kt % 4) * FPC + hi]

        qkts = {}
        v_sbs = {}
        # deferred per-window epilogue: (b, m, p2, rot)
        pending_a = [None]

        def flush_a():
            if pending_a[0] is None:
                return
            b, m, p2, rot = pending_a[0]
            pending_a[0] = None
            # V eviction (frees ps2 for this window's mm256 sweep)
            nc.scalar.copy(v_sbs[b][:, m * HPC * D:(m + 1) * HPC * D], p2[:])
            # transpose rotated q,k into [d, t]; 4 tiles share one bank
            tp4 = pst.tile([P, 4, P], MMDT, tag="tp", name=f"tp_{b}_{m}")
            for hb in range(4):
                nc.tensor.transpose(tp4[:, hb, :], rot[:, hb * P:(hb + 1) * P],
                                    ident[:])
            qv = qkts[b][:].rearrange("p (hb t) -> p hb t", hb=4)
            nc.scalar.copy(qv[:, :, m * P:(m + 1) * P], tp4[:])

        def emit_a(b, m):
            if m == 0:
                qkts[b] = qkt_pool.tile([P, 4 * T], MMDT, tag="qkT",
                                        name=f"qkT_{b}")
                v_sbs[b] = v_pool.tile([P, MT * HPC * D], MMDT, tag="v",
                                       name=f"v_{b}")
            gi = b * MT + m
            # keep two windows of x in flight: the Act queue can lag
            # behind a pair's exp backlog
            for ahead in (1, 2):
                ni = gi + ahead
                if ni < B * MT and (ni // MT, ni % MT) not in prefetched \
                        and ni != gi:
                    prefetch_x(ni // MT, ni % MT)
            xc = fetch_x(b, m)
            p5 = ps5.tile([P, 512], F32, tag="p5", name=f"p5_{b}_{m}")
            p2 = ps2.tile([P, 256], F32, tag="p2", name=f"p2_{b}_{m}")
            for kt in range(KTC):
                nc.tensor.matmul(p5[:], xc[:, kt, :], wslice(kt, 0, 512),
                                 start=(kt == 0), stop=(kt == KTC - 1))
            # previous window's epilogue lands here: its RoPE is done, so
            # the PE transposes never wait, and ps2/pst free up in time
            flush_a()
            # RoPE on the q|k half, writes rot
            gm = b * MT + m
            rot = rotp.tile([P, 512], MMDT, tag="rot", name=f"rot_{b}_{m}")
            p3 = p5[:].rearrange("p (blk two d) -> p blk two d", two=2, d=RD)
            re_, im_ = p3[:, :, 0, :], p3[:, :, 1, :]
            r3 = rot[:].rearrange("p (blk two d) -> p blk two d", two=2, d=RD)
            cosb = (cos_sb[:, gm * RD:(gm + 1) * RD]
                    .unsqueeze(1).broadcast_to([P, 4, RD]))
            sinb = (sin_sb[:, gm * RD:(gm + 1) * RD]
                    .unsqueeze(1).broadcast_to([P, 4, RD]))
            t1 = tmpp.tile([P, 256], F32, tag="t1")
            t2 = tmpp.tile([P, 256], F32, tag="t2")
            t1v = t1[:].rearrange("p (blk d) -> p blk d", d=RD)
            t2v = t2[:].rearrange("p (blk d) -> p blk d", d=RD)
            nc.vector.tensor_tensor(t1v, re_, cosb, MUL)
            nc.vector.tensor_tensor(t2v, im_, sinb, MUL)
            nc.vector.tensor_tensor(r3[:, :, 0, :], t1v, t2v, SUB)
            t3 = tmpp.tile([P, 256], F32, tag="t3")
            t4 = tmpp.tile([P, 256], F32, tag="t4")
            t3v = t3[:].rearrange("p (blk d) -> p blk d", d=RD)
            t4v = t4[:].rearrange("p (blk d) -> p blk d", d=RD)
            nc.vector.tensor_tensor(t3v, re_, sinb, MUL)
            nc.vector.tensor_tensor(t4v, im_, cosb, MUL)
            nc.vector.tensor_tensor(r3[:, :, 1, :], t3v, t4v, ADD)
            # V projection sweep after the epilogue so ps2 (bufs=1) is free
            for kt in range(KTC):
                nc.tensor.matmul(p2[:], xc[:, kt, :], wslice(kt, 512, FPC),
                                 start=(kt == 0), stop=(kt == KTC - 1))
            pending_a[0] = (b, m, p2, rot)

        # ---- attention chunk pairs (both heads, interleaved) ----
        chunk_st = {}

        def emit_p(b, j):
            flush_a()
            qkT = qkts[b]
            v_sb = v_sbs[b]
            nkt = 4 * (j + 1)
            y_ps = {}
            sacc = {}
            for h in range(HPC):
                y_ps[h] = psy.tile([P, NQ], F32, tag="y", name=f"y_{b}_{j}_{h}")
                sacc[h] = saccp.tile([P, NQ], MMDT, tag="sa",
                                     name=f"sa_{b}_{j}_{h}")
            for kt in range(nkt):
                ktl = kt - (nkt - 4)
                qlo = max(ktl, 0) * P
                for h in range(HPC):
                    sc = pss.tile([P, NQ], F32, tag="sc",
                                  name=f"sc_{b}_{j}_{kt}_{h}")
                    nc.tensor.matmul(
                        sc[:, qlo:],
                        qkT[:, (2 + h) * T + kt * P:(2 + h) * T + (kt + 1) * P],
                        qkT[:, h * T + j * NQ + qlo: h * T + (j + 1) * NQ],
                        start=True, stop=True)
                    at = attnp.tile([P, NQ], MMDT, tag="at",
                                    name=f"at_{b}_{j}_{kt}_{h}")
                    nc.scalar.activation(at[:, qlo:], sc[:, qlo:], EXP,
                                         scale=SCALE)
                    if ktl >= 0:
                        nc.vector.tensor_tensor(
                            at[:, qlo:qlo + P], at[:, qlo:qlo + P],
                            msk_sb[:], MUL)
                    if kt == 0:
                        nc.vector.tensor_copy(sacc[h][:], at[:])
                    else:
                        nc.vector.tensor_tensor(sacc[h][:, qlo:],
                                                sacc[h][:, qlo:],
                                                at[:, qlo:], ADD)
                    nc.tensor.matmul(
                        y_ps[h][:, qlo:],
                        v_sb[:, kt * HPC * D + h * D:kt * HPC * D + (h + 1) * D],
                        at[:, qlo:], start=(kt == 0), stop=(kt == nkt - 1),
                        skip_group_check=True)
            chunk_st[(b, j)] = (y_ps, sacc)

        def emit_f1(b, j):
            # denominator partition reduce, issued right after the pair so
            # the 3.5us gpsimd calls drain while the next A windows run
            y_ps, sacc = chunk_st.pop((b, j))
            sreps = {}
            for h in range(HPC):
                srep = srepp.tile([P, NQ], F32, tag="sr",
                                  name=f"sr_{b}_{j}_{h}")
                nc.gpsimd.partition_all_reduce(srep[:], sacc[h][:], P,
                                               bass_isa.ReduceOp.add)
                sreps[h] = srep
            chunk_st[(b, j)] = (y_ps, sreps)

        def emit_f2(b, j):
            # normalization, emitted two windows later: the reduces are
            # done by now so the DVE queue never blocks on gpsimd
            y_ps, sreps = chunk_st.pop((b, j))
            inst0 = b * HPC
            for h in range(HPC):
                rrep = rrepp.tile([P, NQ], F32, tag="rr",
                                  name=f"rr_{b}_{j}_{h}")
                with nc.allow_low_precision(reason="softmax recip"):
                    nc.vector.reciprocal_approx_fast(out=rrep[:],
                                                     in_=sreps[h][:])
                nc.vector.tensor_tensor(
                    yt_all[:, (inst0 + h) * T + j * NQ:
                           (inst0 + h) * T + (j + 1) * NQ],
                    y_ps[h][:], rrep[:], MUL)

        # ---- output projection m-tile (reuses the A/B psum banks) ----
        c_cnt = [0]
        C_POOLS = [(None, "p5"), (None, "sc"), (None, "y")]

        def emit_c(b, m):
            flush_a()
            cpools = [(ps5, "p5"), (pss, "sc"), (psy, "y")]
            orow = outrow.tile([P, C], MMDT, tag="orow")
            for oc in range(4):
                if c_cnt[0] < 8:
                    # early tiles overlap the draining final attention
                    # pair; only ps5 is reliably free then
                    pool, tg = ps5, "p5"
                else:
                    pool, tg = cpools[c_cnt[0] % 3]
                c_cnt[0] += 1
                op = pool.tile([P, 512], F32, tag=tg, name=f"op_{b}_{m}_{oc}")
                for h in range(HPC):
                    nc.tensor.matmul(
                        op[:],
                        yt_all[:, (b * HPC + h) * T + m * P:
                               (b * HPC + h) * T + (m + 1) * P],
                        wp_sb[:, h * C + oc * 512: h * C + (oc + 1) * 512],
                        start=(h == 0), stop=(h == HPC - 1))
                # split each eviction across both engines so the depth-2
                # ps5 rotation never gates the PE
                nc.scalar.copy(orow[:, oc * 512:oc * 512 + 256],
                               op[:, 0:256])
                nc.vector.tensor_copy(orow[:, oc * 512 + 256:(oc + 1) * 512],
                                      op[:, 256:512])
            nc.sync.dma_start(
                out=out_d[(b * MT + m) * P:(b * MT + m + 1) * P, :],
                in_=orow[:])

        # ---- the fused schedule ----
        sched = []
        for b in range(B):
            for m in range(MT):
                sched.append(("A", b, m))
                if b > 0 and m == 0:
                    sched += [("P", b - 1, 3), ("F1", b - 1, 3)]
                if b > 0 and m == 2:
                    sched.append(("F2", b - 1, 3))
                if m in (4, 8, 12):
                    sched += [("P", b, m // 4 - 1), ("F1", b, m // 4 - 1)]
                if m in (6, 10, 14):
                    sched.append(("F2", b, m // 4 - 1))
        bl = B - 1
        sched += [("P", bl, 3), ("F1", bl, 3), ("C", 0, 0), ("F2", bl, 3)]
        for m in range(1, MT):
            sched.append(("C", 0, m))
        for m in range(MT):
            sched.append(("C", 1, m))

        emitters = {"A": emit_a, "P": emit_p, "F1": emit_f1, "F2": emit_f2,
                    "C": emit_c}
        for kind, b, i in sched:
            emitters[kind](b, i)

        wstack.close()

    nc.compile()
    return nc


def _perm(rows):
    return np.concatenate([rows[0::2], rows[1::2]], axis=0)


def _host_inputs(x, mask, freqs_cos, freqs_sin, w_attn, w_proj):
    f32 = np.float32
    f16 = np.float16
    x = np.asarray(x, f32)
    fc = np.asarray(freqs_cos, f32)
    fs = np.asarray(freqs_sin, f32)
    w_attn = np.asarray(w_attn, f32)
    w_proj = np.asarray(w_proj, f32)

    # x in DMA-issue order: per partition, contiguous [b][m][kt][tok]
    Xv = x.reshape(B, MT, P, KTC, P).transpose(4, 0, 1, 3, 2)
    # [p, b, m, kt, tok]
    xt_host = np.ascontiguousarray(Xv.reshape(P, -1)).astype(f16)

    def rows_arrange(a):  # [BT, RD] -> [P, (BT//P)*RD]
        return np.ascontiguousarray(
            a.reshape(BT // P, P, RD).transpose(1, 0, 2).reshape(P, -1))

    cosw = rows_arrange(np.concatenate([fc] * B, axis=0))
    sinw = rows_arrange(np.concatenate([fs] * B, axis=0))

    # one [k, q] triangle (attend iff k <= q) covers every diagonal subtile
    maskd = np.ascontiguousarray(np.triu(np.ones((P, P), dtype=f16)))

    wq, wk, wv = w_attn[0:C], w_attn[C:2 * C], w_attn[2 * C:3 * C]
    in_maps = []
    for c in range(NCORES):
        h0, h1 = HPC * c, HPC * c + 1
        Wc = np.concatenate([
            _perm(wq[h0 * D:(h0 + 1) * D]), _perm(wq[h1 * D:(h1 + 1) * D]),
            _perm(wk[h0 * D:(h0 + 1) * D]), _perm(wk[h1 * D:(h1 + 1) * D]),
            wv[h0 * D:(h0 + 1) * D], wv[h1 * D:(h1 + 1) * D]], axis=0)
        wqkv_c = np.ascontiguousarray(
            Wc.T.reshape(KTC, P, FPC).transpose(1, 0, 2).reshape(P, KTC * FPC)
        ).astype(f16)
        wp_c = w_proj[:, c * HPC * D:(c + 1) * HPC * D].T  # [256, C]
        wp_c = np.ascontiguousarray(
            wp_c.reshape(HPC, P, C).transpose(1, 0, 2).reshape(P, HPC * C)
        ).astype(f16)
        in_maps.append({
            "xt": xt_host, "wqkv": wqkv_c, "cosw": cosw, "sinw": sinw,
            "maskd": maskd, "wproj": wp_c,
        })
    return in_maps


def kernel(x, mask, freqs_cos, freqs_sin, w_attn, w_proj):
    global _PROGRAM
    _ensure_concourse()
    from concourse.bass_utils import run_bass_kernel_spmd

    if _PROGRAM is None:
        _PROGRAM = _build_program()
    nc = _PROGRAM

    in_maps = _host_inputs(x, mask, freqs_cos, freqs_sin, w_attn, w_proj)
    res = run_bass_kernel_spmd(nc, in_maps, list(range(NCORES)))
    out = res.results[0]["outp"].astype(np.float32)
    for i in range(1, NCORES):
        out = out + res.results[i]["outp"].astype(np.float32)
    return np.ascontiguousarray(out.reshape(B, T, C))


# revision 41
# speedup vs baseline: 1.2843x; 1.0512x over previous
"""Causal self-attention (QKV GEMM + RoPE + causal softmax attention + output
projection) for Trainium2, sharded over 8 NeuronCores.

Sharding: tensor-parallel over heads (2 heads/core). Each core computes the
QKV projections for its heads (full token range), RoPE, causal attention, and
a partial output projection over its heads' channels; the host sums the 8
partial projections (the only cross-core reduction) and reshapes.

Matmul operands are fp16 (full-rate PE with hidden weight loads); all
accumulation is fp32 in PSUM, softmax statistics are fp32.

This version fuses the phases into one software-pipelined stream so the
PE never drains between them:
- phase A is emitted as 32 single-m-tile windows (16 QKV k-tiles each);
  RoPE / V-eviction / q,k-transposes of window m are deferred into window
  m+1 so they never stall the PE.
- attention is emitted as head-interleaved query-chunk pairs spliced
  between A windows as soon as their qkT tiles exist; the Scalar-bound
  exp work overlaps the PE-bound GEMM windows.
- softmax denominators: fp16 at-tile adds on DVE, one gpsimd
  partition_all_reduce per chunk (no PE, no extra PSUM), reciprocal and
  scale on DVE, pipelined one A-window behind.
- the output projection is spliced into the attention tail and reuses
  the A/B PSUM pools (everything fits the 8 banks).
- causal narrowing: on diagonal key tiles all ops run only on the valid
  [qlo:] columns, with one shared [128,128] triangle mask.
- all matmul operands are converted to fp16 on the host; x is laid out
  in DMA-issue order (contiguous per partition per window); output
  partials are fp16 and the host accumulates in fp32.
"""

import os
import sys

import numpy as np


def _ensure_concourse():
    try:
        import concourse.bass  # noqa: F401
        return
    except ImportError:
        pass
    for p in (
        "/opt/trn_rl_repo",
        os.path.expanduser("~/.axon_site/_ro/trn_rl_repo"),
        "/root/.axon_site/_ro/trn_rl_repo",
    ):
        if os.path.isdir(p) and p not in sys.path:
            sys.path.insert(0, p)
    import concourse.bass  # noqa: F401


# Problem shape (hardcoded per contract)
B, T, C, H = 2, 2048, 2048, 16
D, RD = 128, 64
NCORES = 8
HPC = H // NCORES          # heads per core = 2
BT = B * T                 # 4096
P = 128
MT = T // P                # 16 token tiles per batch
KTC = C // P               # 16 contraction tiles over C
FPC = 3 * HPC * D          # 768 qkv features per core
NQ = 512                   # query chunk
NJ = T // NQ               # 4 query chunks per instance
SCALE = 1.0 / float(np.sqrt(D))

_PROGRAM = None


def _build_program():
    _ensure_concourse()
    from collections import deque
    from contextlib import ExitStack

    import concourse.bacc as bacc
    import concourse.mybir as mybir
    import concourse.tile as tile
    from concourse import bass_isa
    from concourse.alu_op_type import AluOpType
    from concourse.masks import make_identity

    F32 = mybir.dt.float32
    MMDT = mybir.dt.float16
    EXP = mybir.ActivationFunctionType.Exp
    MUL = AluOpType.mult
    SUB = AluOpType.subtract
    ADD = AluOpType.add
    PSUM = "PSUM"

    nc = bacc.Bacc("TRN2", target_bir_lowering=False, debug=False,
                   num_devices=NCORES)

    xt_d = nc.dram_tensor("xt", [P, BT * KTC], MMDT, kind="ExternalInput").ap()
    w_d = nc.dram_tensor("wqkv", [P, KTC * FPC], MMDT, kind="ExternalInput").ap()
    cos_d = nc.dram_tensor("cosw", [P, (BT // P) * RD], F32, kind="ExternalInput").ap()
    sin_d = nc.dram_tensor("sinw", [P, (BT // P) * RD], F32, kind="ExternalInput").ap()
    msk_d = nc.dram_tensor("maskd", [P, P], MMDT, kind="ExternalInput").ap()
    wp_d = nc.dram_tensor("wproj", [P, HPC * C], MMDT, kind="ExternalInput").ap()
    out_d = nc.dram_tensor("outp", [BT, C], MMDT, kind="ExternalOutput").ap()

    WQ = KTC * FPC // 4        # qkv weight quarter, 4 k-tiles each

    with tile.TileContext(nc) as tc, ExitStack() as gctx:
        ep = gctx.enter_context

        const = ep(tc.tile_pool(name="const", bufs=1))
        msk_sb = const.tile([P, P], MMDT, tag="msk")
        cos_sb = const.tile([P, (BT // P) * RD], F32, tag="cos")
        sin_sb = const.tile([P, (BT // P) * RD], F32, tag="sin")
        ident = const.tile([P, P], MMDT, tag="ident")
        wp_sb = const.tile([P, HPC * C], MMDT, tag="wp")

        make_identity(nc, ident[:])

        qkt_pool = ep(tc.tile_pool(name="qkt", bufs=2))
        v_pool = ep(tc.tile_pool(name="v", bufs=2))
        yt_pool = ep(tc.tile_pool(name="yt", bufs=1))
        yt_all = yt_pool.tile([P, B * HPC * T], MMDT, tag="yt")
        xcol = ep(tc.tile_pool(name="xcol", bufs=4))
        rotp = ep(tc.tile_pool(name="rot", bufs=3))
        tmpp = ep(tc.tile_pool(name="tmp", bufs=2))
        attnp = ep(tc.tile_pool(name="attn", bufs=4))
        saccp = ep(tc.tile_pool(name="sacc", bufs=4))
        srepp = ep(tc.tile_pool(name="srep", bufs=2))
        rrepp = ep(tc.tile_pool(name="rrep", bufs=2))
        outrow = ep(tc.tile_pool(name="orow", bufs=3))

        # PSUM: exactly 8 banks
        ps5 = ep(tc.tile_pool(name="ps5", bufs=2, space=PSUM))   # qk gemm
        ps2 = ep(tc.tile_pool(name="ps2", bufs=1, space=PSUM))   # v gemm
        pst = ep(tc.tile_pool(name="pst", bufs=1, space=PSUM))   # transposes
        pss = ep(tc.tile_pool(name="pss", bufs=2, space=PSUM))   # scores
        psy = ep(tc.tile_pool(name="psy", bufs=2, space=PSUM))   # attn out

        # x chunk prefetching, one [P, KTC, P] chunk per A window
        prefetched = {}

        def fetch_x(b, m):
            key = (b, m)
            if key in prefetched:
                return prefetched.pop(key)
            xo = (b * MT + m) * KTC * P
            xc = xcol.tile([P, KTC, P], MMDT, tag="xc")
            # issue on the Act queue: the gpsimd queue carries the long
            # partition_all_reduce calls and must not delay x loads
            nc.scalar.dma_start(
                out=xc[:],
                in_=xt_d[:, xo:xo + KTC * P].rearrange(
                    "p (k t) -> p k t", k=KTC))
            return xc

        def prefetch_x(b, m):
            prefetched[(b, m)] = fetch_x(b, m)

        prefetch_x(0, 0)

        wstack = ExitStack()
        wpool = wstack.enter_context(tc.tile_pool(name="wqkv", bufs=1))
        w_sbs = [wpool.tile([P, WQ], MMDT, tag=f"w{q}", name=f"w{q}")
                 for q in range(4)]
        for q in range(4):
            nc.sync.dma_start(out=w_sbs[q][:], in_=w_d[:, q * WQ:(q + 1) * WQ])
        # consts after the weights on the SP queue: RoPE needs cos/sin only
        # at the first window's tail, the mask at the first pair
        nc.sync.dma_start(out=cos_sb[:], in_=cos_d)
        nc.sync.dma_start(out=sin_sb[:], in_=sin_d)
        nc.sync.dma_start(out=msk_sb[:], in_=msk_d)
        nc.sync.dma_start(out=wp_sb[:], in_=wp_d)

        def wslice(kt, lo, hi):
            return w_sbs[kt // 4][:, (kt % 4) * FPC + lo:(kt % 4) * FPC + hi]

        qkts = {}
        v_sbs = {}
        # attention work broken into per-tile quanta, pumped one per
        # matmul slot inside the A windows / C tiles so the Scalar-bound
        # exp stream overlaps the PE-bound GEMMs instead of serializing
        quanta = deque()

        def pump(n=1):
            for _ in range(n):
                if not quanta:
                    return
                quanta.popleft()()

        # deferred per-window epilogue: (b, m, p2, rot)
        pending_a = [None]

        def flush_a():
            if pending_a[0] is None:
                return
            b, m, p2, rot = pending_a[0]
            pending_a[0] = None
            # V eviction (frees ps2 for this window's mm256 sweep)
            nc.scalar.copy(v_sbs[b][:, m * HPC * D:(m + 1) * HPC * D], p2[:])
            # transpose rotated q,k into [d, t]; 4 tiles share one bank
            tp4 = pst.tile([P, 4, P], MMDT, tag="tp", name=f"tp_{b}_{m}")
            for hb in range(4):
                nc.tensor.transpose(tp4[:, hb, :], rot[:, hb * P:(hb + 1) * P],
                                    ident[:])
            qv = qkts[b][:].rearrange("p (hb t) -> p hb t", hb=4)
            nc.scalar.copy(qv[:, :, m * P:(m + 1) * P], tp4[:])

        def emit_a(b, m):
            if m == 0:
                qkts[b] = qkt_pool.tile([P, 4 * T], MMDT, tag="qkT",
                                        name=f"qkT_{b}")
                v_sbs[b] = v_pool.tile([P, MT * HPC * D], MMDT, tag="v",
                                       name=f"v_{b}")
            gi = b * MT + m
            # keep two windows of x in flight: the Act queue can lag
            # behind a pair's exp backlog
            for ahead in (1, 2):
                ni = gi + ahead
                if ni < B * MT and (ni // MT, ni % MT) not in prefetched \
                        and ni != gi:
                    prefetch_x(ni // MT, ni % MT)
            xc = fetch_x(b, m)
            p5 = ps5.tile([P, 512], F32, tag="p5", name=f"p5_{b}_{m}")
            p2 = ps2.tile([P, 256], F32, tag="p2", name=f"p2_{b}_{m}")
            for kt in range(KTC):
                nc.tensor.matmul(p5[:], xc[:, kt, :], wslice(kt, 0, 512),
                                 start=(kt == 0), stop=(kt == KTC - 1))
                pump()
            # previous window's epilogue lands here: its RoPE is done, so
            # the PE transposes never wait, and ps2/pst free up in time
            flush_a()
            # RoPE on the q|k half, writes rot
            gm = b * MT + m
            rot = rotp.tile([P, 512], MMDT, tag="rot", name=f"rot_{b}_{m}")
            p3 = p5[:].rearrange("p (blk two d) -> p blk two d", two=2, d=RD)
            re_, im_ = p3[:, :, 0, :], p3[:, :, 1, :]
            r3 = rot[:].rearrange("p (blk two d) -> p blk two d", two=2, d=RD)
            cosb = (cos_sb[:, gm * RD:(gm + 1) * RD]
                    .unsqueeze(1).broadcast_to([P, 4, RD]))
            sinb = (sin_sb[:, gm * RD:(gm + 1) * RD]
                    .unsqueeze(1).broadcast_to([P, 4, RD]))
            t1 = tmpp.tile([P, 256], F32, tag="t1")
            t2 = tmpp.tile([P, 256], F32, tag="t2")
            t1v = t1[:].rearrange("p (blk d) -> p blk d", d=RD)
            t2v = t2[:].rearrange("p (blk d) -> p blk d", d=RD)
            nc.vector.tensor_tensor(t1v, re_, cosb, MUL)
            nc.vector.tensor_tensor(t2v, im_, sinb, MUL)
            nc.vector.tensor_tensor(r3[:, :, 0, :], t1v, t2v, SUB)
            t3 = tmpp.tile([P, 256], F32, tag="t3")
            t4 = tmpp.tile([P, 256], F32, tag="t4")
            t3v = t3[:].rearrange("p (blk d) -> p blk d", d=RD)
            t4v = t4[:].rearrange("p (blk d) -> p blk d", d=RD)
            nc.vector.tensor_tensor(t3v, re_, sinb, MUL)
            nc.vector.tensor_tensor(t4v, im_, cosb, MUL)
            nc.vector.tensor_tensor(r3[:, :, 1, :], t3v, t4v, ADD)
            # V projection sweep after the epilogue so ps2 (bufs=1) is free
            for kt in range(KTC):
                nc.tensor.matmul(p2[:], xc[:, kt, :], wslice(kt, 512, FPC),
                                 start=(kt == 0), stop=(kt == KTC - 1))
                pump()
            pending_a[0] = (b, m, p2, rot)

        # ---- attention chunk pairs (both heads), as pumpable quanta ----
        chunk_st = {}

        def enqueue_p(b, j):
            qkT = qkts[b]
            v_sb = v_sbs[b]
            nkt = 4 * (j + 1)
            st = {"y": {}, "sacc": {}, "at": {}}
            chunk_st[(b, j)] = st

            def q_score(kt, h, ktl, qlo):
                def f():
                    if kt == 0:
                        st["y"][h] = psy.tile([P, NQ], F32, tag="y",
                                              name=f"y_{b}_{j}_{h}")
                        st["sacc"][h] = saccp.tile([P, NQ], MMDT, tag="sa",
                                                   name=f"sa_{b}_{j}_{h}")
                    sacc = st["sacc"][h]
                    sc = pss.tile([P, NQ], F32, tag="sc",
                                  name=f"sc_{b}_{j}_{kt}_{h}")
                    nc.tensor.matmul(
                        sc[:, qlo:],
                        qkT[:, (2 + h) * T + kt * P:
                            (2 + h) * T + (kt + 1) * P],
                        qkT[:, h * T + j * NQ + qlo: h * T + (j + 1) * NQ],
                        start=True, stop=True)
                    at = attnp.tile([P, NQ], MMDT, tag="at",
                                    name=f"at_{b}_{j}_{kt}_{h}")
                    nc.scalar.activation(at[:, qlo:], sc[:, qlo:], EXP,
                                         scale=SCALE)
                    if ktl >= 0:
                        nc.vector.tensor_tensor(
                            at[:, qlo:qlo + P], at[:, qlo:qlo + P],
                            msk_sb[:], MUL)
                    if kt == 0:
                        nc.vector.tensor_copy(sacc[:], at[:])
                    else:
                        nc.vector.tensor_tensor(sacc[:, qlo:], sacc[:, qlo:],
                                                at[:, qlo:], ADD)
                    st["at"][(kt, h)] = at
                return f

            def q_v(kt, h, qlo):
                def f():
                    at = st["at"].pop((kt, h))
                    nc.tensor.matmul(
                        st["y"][h][:, qlo:],
                        v_sb[:, kt * HPC * D + h * D:
                             kt * HPC * D + (h + 1) * D],
                        at[:, qlo:], start=(kt == 0), stop=(kt == nkt - 1),
                        skip_group_check=True)
                return f

            for kt in range(nkt):
                ktl = kt - (nkt - 4)
                qlo = max(ktl, 0) * P
                for h in range(HPC):
                    quanta.append(q_score(kt, h, ktl, qlo))
                for h in range(HPC):
                    quanta.append(q_v(kt, h, qlo))

        def emit_f1(b, j):
            # denominator partition reduce, issued once the pair's quanta
            # have all been emitted; the 3.5us gpsimd calls drain while
            # the next A windows run (nothing else shares that queue)
            pump(len(quanta))
            st = chunk_st[(b, j)]
            st["srep"] = {}
            for h in range(HPC):
                srep = srepp.tile([P, NQ], F32, tag="sr",
                                  name=f"sr_{b}_{j}_{h}")
                nc.gpsimd.partition_all_reduce(srep[:], st["sacc"][h][:], P,
                                               bass_isa.ReduceOp.add)
                st["srep"][h] = srep

        def emit_f2(b, j):
            # normalization, emitted two windows later: the reduces are
            # done by now so the DVE queue never blocks on gpsimd
            st = chunk_st.pop((b, j))
            inst0 = b * HPC
            for h in range(HPC):
                rrep = rrepp.tile([P, NQ], F32, tag="rr",
                                  name=f"rr_{b}_{j}_{h}")
                with nc.allow_low_precision(reason="softmax recip"):
                    nc.vector.reciprocal_approx_fast(out=rrep[:],
                                                     in_=st["srep"][h][:])
                nc.vector.tensor_tensor(
                    yt_all[:, (inst0 + h) * T + j * NQ:
                           (inst0 + h) * T + (j + 1) * NQ],
                    st["y"][h][:], rrep[:], MUL)

        # ---- output projection m-tile (reuses the A/B psum banks) ----
        c_cnt = [0]

        def emit_c(b, m):
            flush_a()
            orow = outrow.tile([P, C], MMDT, tag="orow")
            for oc in range(4):
                pool, tg = (ps5, "p5") if c_cnt[0] % 2 == 0 else (pss, "sc")
                c_cnt[0] += 1
                op = pool.tile([P, 512], F32, tag=tg, name=f"op_{b}_{m}_{oc}")
                for h in range(HPC):
                    nc.tensor.matmul(
                        op[:],
                        yt_all[:, (b * HPC + h) * T + m * P:
                               (b * HPC + h) * T + (m + 1) * P],
                        wp_sb[:, h * C + oc * 512: h * C + (oc + 1) * 512],
                        start=(h == 0), stop=(h == HPC - 1))
                pump()
                # split each eviction across both engines so the depth-4
                # rotation never gates the PE
                nc.scalar.copy(orow[:, oc * 512:oc * 512 + 256],
                               op[:, 0:256])
                nc.vector.tensor_copy(orow[:, oc * 512 + 256:(oc + 1) * 512],
                                      op[:, 256:512])
            nc.sync.dma_start(
                out=out_d[(b * MT + m) * P:(b * MT + m + 1) * P, :],
                in_=orow[:])

        # ---- the fused schedule ----
        # pair j's quanta enqueue after window 4j+4 (its qkT tiles flush
        # in window 4j+4) and drain inside windows 4j+5.. ; F1 two windows
        # later force-drains leftovers, F2 two more windows later.
        sched = []
        for b in range(B):
            for m in range(MT):
                sched.append(("A", b, m))
                if b > 0:
                    if m == 0:
                        # F2(b-1,2) must precede window 1's pumps: the
                        # j3 pair's psum-y rotation reuses its buffers
                        sched += [("P", b - 1, 3), ("F2", b - 1, 2)]
                    elif m == 2:
                        sched.append(("F1", b - 1, 3))
                    elif m == 4:
                        sched.append(("F2", b - 1, 3))
                if m in (4, 8, 12):
                    sched.append(("P", b, m // 4 - 1))
                if m in (6, 10, 14):
                    sched.append(("F1", b, m // 4 - 1))
                if m in (8, 12):
                    sched.append(("F2", b, m // 4 - 2))
        bl = B - 1
        sched += [("F2", bl, 2), ("P", bl, 3)]
        for m in range(MT):
            sched.append(("C", 0, m))
        sched.append(("F1", bl, 3))
        sched += [("C", 1, 0), ("C", 1, 1)]
        sched.append(("F2", bl, 3))
        for m in range(2, MT):
            sched.append(("C", 1, m))

        emitters = {"A": emit_a, "P": enqueue_p, "F1": emit_f1,
                    "F2": emit_f2, "C": emit_c}
        for kind, b, i in sched:
            emitters[kind](b, i)
        pump(len(quanta))

        wstack.close()

    nc.compile()
    return nc


def _perm(rows):
    return np.concatenate([rows[0::2], rows[1::2]], axis=0)


def _host_inputs(x, mask, freqs_cos, freqs_sin, w_attn, w_proj):
    f32 = np.float32
    f16 = np.float16
    x = np.asarray(x, f32)
    fc = np.asarray(freqs_cos, f32)
    fs = np.asarray(freqs_sin, f32)
    w_attn = np.asarray(w_attn, f32)
    w_proj = np.asarray(w_proj, f32)

    # x in DMA-issue order: per partition, contiguous [b][m][kt][tok]
    Xv = x.reshape(B, MT, P, KTC, P).transpose(4, 0, 1, 3, 2)
    # [p, b, m, kt, tok]
    xt_host = np.ascontiguousarray(Xv.reshape(P, -1)).astype(f16)

    def rows_arrange(a):  # [BT, RD] -> [P, (BT//P)*RD]
        return np.ascontiguousarray(
            a.reshape(BT // P, P, RD).transpose(1, 0, 2).reshape(P, -1))

    cosw = rows_arrange(np.concatenate([fc] * B, axis=0))
    sinw = rows_arrange(np.concatenate([fs] * B, axis=0))

    # one [k, q] triangle (attend iff k <= q) covers every diagonal subtile
    maskd = np.ascontiguousarray(np.triu(np.ones((P, P), dtype=f16)))

    wq, wk, wv = w_attn[0:C], w_attn[C:2 * C], w_attn[2 * C:3 * C]
    in_maps = []
    for c in range(NCORES):
        h0, h1 = HPC * c, HPC * c + 1
        Wc = np.concatenate([
            _perm(wq[h0 * D:(h0 + 1) * D]), _perm(wq[h1 * D:(h1 + 1) * D]),
            _perm(wk[h0 * D:(h0 + 1) * D]), _perm(wk[h1 * D:(h1 + 1) * D]),
            wv[h0 * D:(h0 + 1) * D], wv[h1 * D:(h1 + 1) * D]], axis=0)
        wqkv_c = np.ascontiguousarray(
            Wc.T.reshape(KTC, P, FPC).transpose(1, 0, 2).reshape(P, KTC * FPC)
        ).astype(f16)
        wp_c = w_proj[:, c * HPC * D:(c + 1) * HPC * D].T  # [256, C]
        wp_c = np.ascontiguousarray(
            wp_c.reshape(HPC, P, C).transpose(1, 0, 2).reshape(P, HPC * C)
        ).astype(f16)
        in_maps.append({
            "xt": xt_host, "wqkv": wqkv_c, "cosw": cosw, "sinw": sinw,
            "maskd": maskd, "wproj": wp_c,
        })
    return in_maps


def kernel(x, mask, freqs_cos, freqs_sin, w_attn, w_proj):
    global _PROGRAM
    _ensure_concourse()
    from concourse.bass_utils import run_bass_kernel_spmd

    if _PROGRAM is None:
        _PROGRAM = _build_program()
    nc = _PROGRAM

    in_maps = _host_inputs(x, mask, freqs_cos, freqs_sin, w_attn, w_proj)
    res = run_bass_kernel_spmd(nc, in_maps, list(range(NCORES)))
    out = res.results[0]["outp"].astype(np.float32)
    for i in range(1, NCORES):
        out = out + res.results[i]["outp"].astype(np.float32)
    return np.ascontiguousarray(out.reshape(B, T, C))
